# revision 4
# baseline (speedup 1.0000x reference)
"""StyleGAN2-style G-block on 8 Trainium2 NeuronCores (batch-parallel, 1 sample/core).

Strategy
--------
- Modulation (x * style) and demodulation (/sigma) are folded into per-sample
  effective weights on the host (each core owns one sample, so weights are
  per-core constants).
- The stride-2 transposed conv decomposes into 4 output-parity classes, each a
  small conv over the 128x128 input grid (taps: ee=1, eo=2, oe=2, oo=4).
- Both convs run on the PE as fp32r matmuls (channels on partitions, pixels on
  the moving free dim, taps accumulated in PSUM).
- noise * ns is added with a rank-1 matmul (ones[1,M].T @ noise[1,N]) into the
  same PSUM accumulation; bias + leaky-relu(0.2) are applied by one ScalarE
  Prelu activation per tile during PSUM evacuation.
- conv2 (3x3) consumes the upconv activations straight from SBUF (row-block
  pipeline with 1-row halo across blocks); h2 is written to HBM and also kept
  in SBUF to feed the 1x1 to-RGB matmul.  The bilinear skip upsample of y is
  precomputed on the host (it is input-only).
"""

import numpy as np

import concourse.bacc as bacc
import concourse.mybir as mybir
import concourse.tile as tile
from concourse.bass_utils import run_bass_kernel_spmd

F32 = mybir.dt.float32
F32R = mybir.dt.float32r
AF = mybir.ActivationFunctionType

N, CIN, CINT, COUT, KK, L, H, W = 8, 256, 192, 128, 3, 512, 128, 128
H2, W2 = 2 * H, 2 * W
EPS = 1e-8
NBLK = 32          # row blocks; each = 4 x-rows -> 8 h/h2 rows
XROWS = H // NBLK  # 4
HROWS = 2 * XROWS  # 8
XW = W + 1         # padded x cols (right/bottom zero pad)
HW = W2 + 2        # padded h cols (left+right zero pad)

# upconv tap tables: out[2r+a, 2c+b] += x[r+dr, c+dc] * w[:, :, ky, kx]
def _cls_taps(a, b):
    kys = ((1, 0),) if a == 0 else ((0, 1), (2, 0))   # (ky, dr)
    kxs = ((1, 0),) if b == 0 else ((0, 1), (2, 0))   # (kx, dc)
    return [(ky, kx, dr, dc) for (ky, dr) in kys for (kx, dc) in kxs]


def build_bass():
    nc = bacc.Bacc("TRN2", target_bir_lowering=False, debug=False, num_devices=N)
    dp = nc.declare_dram_parameter
    xp = dp("xp", [CIN, (H + 1) * XW], F32, isOutput=False)
    wup = dp("wup", [128, 2 * 9 * CINT], F32, isOutput=False)
    wc0 = dp("wc0", [128, 9 * COUT], F32, isOutput=False)
    wc1 = dp("wc1", [64, 9 * COUT], F32, isOutput=False)
    wrgb = dp("wrgb", [128, 3], F32, isOutput=False)
    n1c = dp("n1c", [1, 4 * H * W], F32, isOutput=False)
    n2s = dp("n2s", [1, H2 * W2], F32, isOutput=False)
    yup = dp("yup", [3, H2 * W2], F32, isOutput=False)
    bup0 = dp("bup0", [128, 1], F32, isOutput=False)
    bup1 = dp("bup1", [64, 1], F32, isOutput=False)
    bcv = dp("bcv", [128, 1], F32, isOutput=False)
    brgb = dp("brgb", [3, 1], F32, isOutput=False)
    onesd = dp("onesd", [1, 128], F32, isOutput=False)
    h2o = dp("h2o", [COUT, H2 * W2], F32, isOutput=True)
    yo = dp("yo", [3, H2 * W2], F32, isOutput=True)

    r = F32R
    with tile.TileContext(nc) as tc:
        with (
            tc.tile_pool(name="const", bufs=1) as cpool,
            tc.tile_pool(name="xp", bufs=2) as xpool,
            tc.tile_pool(name="np", bufs=2) as npool,
            tc.tile_pool(name="ha", bufs=4) as hapool,
            tc.tile_pool(name="h2", bufs=2) as h2pool,
            tc.tile_pool(name="yu", bufs=2) as yupool,
            tc.tile_pool(name="ry", bufs=3) as rypool,
            tc.tile_pool(name="pu", bufs=4, space="PSUM") as pupool,
            tc.tile_pool(name="pc", bufs=3, space="PSUM") as pcpool,
            tc.tile_pool(name="pr", bufs=1, space="PSUM") as prpool,
        ):
            # constants
            wupt = cpool.tile([128, 2 * 9 * CINT], F32)
            wc0t = cpool.tile([128, 9 * COUT], F32)
            wc1t = cpool.tile([64, 9 * COUT], F32)
            wrgbt = cpool.tile([128, 3], F32)
            bup0t = cpool.tile([128, 1], F32)
            bup1t = cpool.tile([64, 1], F32)
            bcvt = cpool.tile([128, 1], F32)
            brgbt = cpool.tile([3, 1], F32)
            alt = cpool.tile([128, 1], F32)
            onest = cpool.tile([1, 128], F32)
            nc.sync.dma_start(wupt[:].bitcast(r), wup[:].bitcast(r))
            nc.sync.dma_start(wc0t[:].bitcast(r), wc0[:].bitcast(r))
            nc.sync.dma_start(wc1t[:].bitcast(r), wc1[:].bitcast(r))
            nc.sync.dma_start(wrgbt[:].bitcast(r), wrgb[:].bitcast(r))
            nc.sync.dma_start(bup0t[:], bup0[:])
            nc.sync.dma_start(bup1t[:], bup1[:])
            nc.sync.dma_start(bcvt[:], bcv[:])
            nc.sync.dma_start(brgbt[:], brgb[:])
            nc.vector.memset(alt[:], 0.2)
            nc.sync.dma_start(onest[:].bitcast(r), onesd[:].bitcast(r))

            has = {}   # block -> (ha0, ha1)

            def upconv_block(k):
                r0 = XROWS * k
                xt0 = xpool.tile([128, 5 * XW], F32, tag="xt0")
                xt1 = xpool.tile([128, 5 * XW], F32, tag="xt1")
                nc.sync.dma_start(xt0[:].bitcast(r),
                                  xp[0:128, r0 * XW:(r0 + 5) * XW].bitcast(r))
                nc.sync.dma_start(xt1[:].bitcast(r),
                                  xp[128:256, r0 * XW:(r0 + 5) * XW].bitcast(r))
                n1t = npool.tile([1, 4 * 512], F32, tag="n1t")
                n1_in = n1c[0:1, :].rearrange("p (c l) -> p c l", c=4)[:, :, k * 512:(k + 1) * 512]
                nc.sync.dma_start(
                    n1t[0:1, :].rearrange("p (c l) -> p c l", c=4).bitcast(r),
                    n1_in.bitcast(r))

                ha0 = hapool.tile([128, HROWS * HW], F32, tag="ha0")
                ha1 = hapool.tile([64, HROWS * HW], F32, tag="ha1")
                has[k] = (ha0, ha1)
                # zero the column pads (positions 0 and HW-1 of each row)
                nc.vector.memset(
                    ha0[:, :].rearrange("p (q c) -> p q c", q=HROWS)[:, :, 0:HW:HW - 1], 0.0)
                nc.vector.memset(
                    ha1[:, :].rearrange("p (q c) -> p q c", q=HROWS)[:, :, 0:HW:HW - 1], 0.0)

                xts = (xt0, xt1)
                for cls in range(4):
                    a, b = divmod(cls, 2)
                    taps = _cls_taps(a, b)
                    for mt in range(2):
                        M = 128 if mt == 0 else 64
                        pt = pupool.tile([128, 512], F32, tag="pu")
                        first = True
                        for kt in range(2):
                            xv = xts[kt][:, :].rearrange("p (q c) -> p q c", q=5)
                            for (ky, kx, dr, dc) in taps:
                                lo = kt * (9 * CINT) + (ky * 3 + kx) * CINT + mt * 128
                                nc.tensor.matmul(
                                    pt[:M, :],
                                    wupt[:, lo:lo + M].bitcast(r),
                                    xv[:, dr:dr + 4, dc:dc + W].bitcast(r),
                                    start=first, stop=False)
                                first = False
                        nc.tensor.matmul(
                            pt[:M, :], onest[0:1, :M].bitcast(r),
                            n1t[0:1, cls * 512:(cls + 1) * 512].bitcast(r),
                            start=False, stop=True)
                        dest = ha0 if mt == 0 else ha1
                        bias = bup0t if mt == 0 else bup1t
                        dap = dest[:M, :].rearrange("p (q c) -> p q c", q=HROWS)
                        dap = dap[:, a:HROWS:2, b + 1:b + 1 + 2 * W:2]
                        nc.scalar.activation(
                            dap.bitcast(r),
                            pt[:M, :].rearrange("p (q c) -> p q c", q=4),
                            AF.Prelu, bias=bias[:M], scale=1.0, alpha=alt[:M])

            def conv_block(b, last):
                q0 = HROWS * b
                n2t = npool.tile([1, HROWS * W2], F32, tag="n2t")
                nc.sync.dma_start(n2t[0:1, :].bitcast(r),
                                  n2s[0:1, q0 * W2:(q0 + HROWS) * W2].bitcast(r))
                h2t = h2pool.tile([128, HROWS * W2], F32, tag="h2t")
                prev = has.get(b - 1)
                cur = has[b]
                nxt = None if last else has[b + 1]
                wcts = (wc0t, wc1t)
                # tap order: dy=0 first so the first matmul covers the full chunk
                taps = [(0, -1), (0, 0), (0, 1),
                        (-1, -1), (-1, 0), (-1, 1), (1, -1), (1, 0), (1, 1)]
                for g in range(2):
                    pts = []
                    for j in range(2):
                        pts.append(pcpool.tile([128, 512], F32, tag="pc", name=f"pc_{b}_{g}_{j}"))
                    started = [False, False]
                    for kt in range(2):
                        K = 128 if kt == 0 else 64
                        wct = wcts[kt]
                        for (dy, dx) in taps:
                            t = (dy + 1) * 3 + (dx + 1)
                            lhsT = wct[:K, t * COUT:t * COUT + COUT].bitcast(r)
                            for j in range(2):
                                lr = 4 * g + 2 * j        # local first row of chunk
                                rows = (lr + dy, lr + 1 + dy)  # local h rows needed
                                # segments: list of (tile, local_row, psum_col)
                                segs = []
                                if rows[0] < 0:
                                    if prev is not None:
                                        segs.append((prev, HROWS - 1, 0))
                                    segs.append((cur, 0, W2))
                                elif rows[1] > HROWS - 1:
                                    segs.append((cur, HROWS - 1, 0))
                                    if nxt is not None:
                                        segs.append((nxt, 0, W2))
                                else:
                                    segs.append((cur, rows[0], None))
                                for (tiles, lrow, pcol) in segs:
                                    src = tiles[kt][:K, :].rearrange(
                                        "p (q c) -> p q c", q=HROWS)
                                    if pcol is None:
                                        rhs = src[:, lrow:lrow + 2, dx + 1:dx + 1 + W2]
                                        out = pts[j][:, :]
                                        nn = 512
                                    else:
                                        rhs = src[:, lrow:lrow + 1, dx + 1:dx + 1 + W2]
                                        out = pts[j][:, pcol:pcol + W2]
                                        nn = W2
                                    nc.tensor.matmul(out, lhsT, rhs.bitcast(r),
                                                     start=not started[j], stop=False)
                                    started[j] = True
                    for j in range(2):
                        lr = 4 * g + 2 * j
                        nc.tensor.matmul(
                            pts[j][:, :], onest[0:1, :COUT].bitcast(r),
                            n2t[0:1, lr * W2:lr * W2 + 512].bitcast(r),
                            start=False, stop=True)
                        nc.scalar.activation(
                            h2t[:, lr * W2:lr * W2 + 512].bitcast(r), pts[j][:, :],
                            AF.Prelu, bias=bcvt[:], scale=1.0, alpha=alt[:])
                # to-RGB + skip
                yupt = yupool.tile([3, HROWS * W2], F32, tag="yupt")
                nc.sync.dma_start(yupt[:], yup[:, q0 * W2:(q0 + HROWS) * W2])
                for j4 in range(4):
                    pr = prpool.tile([3, 512], F32, tag="pr")
                    nc.tensor.matmul(pr[:3, :], wrgbt[:, 0:3].bitcast(r),
                                     h2t[:, j4 * 512:(j4 + 1) * 512].bitcast(r),
                                     start=True, stop=True)
                    ryt = rypool.tile([3, 512], F32, tag="ryt", name=f"ry_{b}_{j4}")
                    nc.scalar.activation(ryt[:], pr[:3, :],
                                         AF.Prelu, bias=brgbt[:], scale=1.0,
                                         alpha=alt[:3])
                    nc.vector.tensor_add(ryt[:], ryt[:],
                                         yupt[:, j4 * 512:(j4 + 1) * 512])
                    nc.sync.dma_start(yo[:, q0 * W2 + j4 * 512:
                                         q0 * W2 + (j4 + 1) * 512], ryt[:])
                nc.sync.dma_start(h2o[:, q0 * W2:(q0 + HROWS) * W2], h2t[:])

            for k in range(NBLK):
                upconv_block(k)
                if k >= 1:
                    conv_block(k - 1, last=False)
            conv_block(NBLK - 1, last=True)

    nc.compile()
    return nc


_NC_CACHE = None


def _get_nc():
    global _NC_CACHE
    if _NC_CACHE is None:
        _NC_CACHE = build_bass()
    return _NC_CACHE


def _bilinear_up2(y):
    # [C,H,W] -> [C,2H,2W], half-pixel centers, edge clamp
    def up_rows(a):
        lo = np.concatenate([a[:, :1, :], a[:, :-1, :]], axis=1)
        hi = np.concatenate([a[:, 1:, :], a[:, -1:, :]], axis=1)
        out = np.empty((a.shape[0], 2 * a.shape[1], a.shape[2]), np.float32)
        out[:, 0::2, :] = 0.25 * lo + 0.75 * a
        out[:, 1::2, :] = 0.75 * a + 0.25 * hi
        return out
    t = up_rows(y.astype(np.float32))
    t = up_rows(t.transpose(0, 2, 1)).transpose(0, 2, 1)
    return t


def _prep_core(n, x, v, y, noise1, noise2, w_up, b_up, sw_up, sb_up,
               w_conv, b_conv, sw_conv, sb_conv, w_rgb, b_rgb, sw_rgb, sb_rgb,
               ns1, ns2):
    f32 = np.float32
    s_up = (v[n] @ sw_up.T + sb_up + 1.0).astype(np.float64)
    wsq_up = np.sum(w_up.astype(np.float64) ** 2, (2, 3)).T          # [o,i]
    sig_up = np.sqrt(wsq_up @ (s_up ** 2) + EPS)                     # [192]
    wup_eff = (w_up * s_up[:, None, None, None].astype(f32)
               / sig_up[None, :, None, None].astype(f32)).astype(f32)  # [i,o,ky,kx]
    s_c = (v[n] @ sw_conv.T + sb_conv + 1.0).astype(np.float64)
    wsq_c = np.sum(w_conv.astype(np.float64) ** 2, (2, 3))           # [o,i]
    sig_c = np.sqrt(wsq_c @ (s_c ** 2) + EPS)                        # [128]
    wc_eff = (w_conv * s_c[None, :, None, None].astype(f32)
              / sig_c[:, None, None, None].astype(f32)).astype(f32)  # [o,i,ky,kx]
    s_r = (v[n] @ sw_rgb.T + sb_rgb + 1.0).astype(f32)
    wr_eff = (w_rgb[:, :, 0, 0] * s_r[None, :]).astype(f32)          # [3,128]

    xpad = np.zeros((CIN, H + 1, XW), f32)
    xpad[:, :H, :W] = x[n]
    wup_h = wup_eff.reshape(2, 128, CINT, 3, 3).transpose(1, 0, 3, 4, 2) \
        .reshape(128, 2 * 9 * CINT).copy()
    wc_h = wc_eff.transpose(1, 2, 3, 0).reshape(CINT, 9 * COUT)
    n1 = (np.float32(ns1) * noise1[n, 0]).astype(f32)
    n1c = np.stack([n1[a::2, b::2].reshape(-1) for a in (0, 1) for b in (0, 1)])
    n2 = (np.float32(ns2) * noise2[n, 0]).astype(f32)
    return {
        "xp": xpad.reshape(CIN, -1),
        "wup": wup_h,
        "wc0": wc_h[:128].copy(),
        "wc1": wc_h[128:].copy(),
        "wrgb": wr_eff.T.copy(),
        "n1c": n1c.reshape(1, -1),
        "n2s": n2.reshape(1, -1),
        "yup": _bilinear_up2(y[n]).reshape(3, -1),
        "bup0": b_up[:128].reshape(128, 1).astype(f32),
        "bup1": b_up[128:].reshape(64, 1).astype(f32),
        "bcv": b_conv.reshape(128, 1).astype(f32),
        "brgb": b_rgb.reshape(3, 1).astype(f32),
        "onesd": np.ones((1, 128), f32),
    }


def kernel(x, v, y, noise1, noise2, w_up, b_up, sw_up, sb_up,
           w_conv, b_conv, sw_conv, sb_conv, w_rgb, b_rgb, sw_rgb, sb_rgb,
           ns1, ns2):
    args = dict(x=np.asarray(x, np.float32), v=np.asarray(v, np.float32),
                y=np.asarray(y, np.float32),
                noise1=np.asarray(noise1, np.float32),
                noise2=np.asarray(noise2, np.float32),
                w_up=np.asarray(w_up, np.float32), b_up=np.asarray(b_up, np.float32),
                sw_up=np.asarray(sw_up, np.float32), sb_up=np.asarray(sb_up, np.float32),
                w_conv=np.asarray(w_conv, np.float32), b_conv=np.asarray(b_conv, np.float32),
                sw_conv=np.asarray(sw_conv, np.float32), sb_conv=np.asarray(sb_conv, np.float32),
                w_rgb=np.asarray(w_rgb, np.float32), b_rgb=np.asarray(b_rgb, np.float32),
                sw_rgb=np.asarray(sw_rgb, np.float32), sb_rgb=np.asarray(sb_rgb, np.float32),
                ns1=np.float32(ns1), ns2=np.float32(ns2))
    in_maps = [_prep_core(n, **args) for n in range(N)]
    nc = _get_nc()
    res = run_bass_kernel_spmd(nc, in_maps, list(range(N)))
    h2 = np.stack([res.results[n]["h2o"].reshape(COUT, H2, W2) for n in range(N)])
    yout = np.stack([res.results[n]["yo"].reshape(3, H2, W2) for n in range(N)])
    return h2, yout
